# revision 3
# baseline (speedup 1.0000x reference)
"""DLRM dot-interaction kernel for Trainium2 (8 NeuronCores, batch-sharded).

Per sample b: T = concat(dense[b], embs[b]) -> [27, 128]; Z = T @ T^T;
output = strict upper triangle of Z -> [351] fp32.

Per-core plan (2048 samples, 16 blocks of 128), v2:
  - SWDGE cast-DMA loads input blocks as [128 b, (f,d)] fp16.
  - PE transposes each [128 b, 128 d] feature slab into PSUM; DVE/ACT copy
    into f-major Tt [128 d, f*128+b] fp16.
  - Packed Gram matmuls: per 4-sample group one LDWEIGHTS (128 cols =
    4 samples x 32-feature pitch from Tt via strided AP) + one 108-col
    matmul (4 samples x 27 features) -> PSUM [128, 108-of-128]:
    diagonal 27x27 blocks (g==g') are the per-sample Gram matrices.
  - DVE/ACT copy the diag blocks to SBUF Zs [(g,m) part, (blk,q,n)] fp32.
  - Triu pack straight to DRAM: per quarter (4 blocks) 26 HWDGE DMAs (one
    per m) scatter z[m, m+1:27] runs into out[s, off_m:off_m+26-m]; no
    DRAM scratch bounce.
"""

import numpy as np

B, NUM_EMBS, D = 16384, 26, 128
N_CORES = 8
BC = B // N_CORES  # 2048 samples per core
BLK = 128          # samples per block
NF = NUM_EMBS + 1  # 27 features
FP = 32            # feature pitch in Tt (27 + 5 junk pad slots)
NPAIR = NF * (NF - 1) // 2  # 351

_CACHE = {}


def build(bc=BC):
    import concourse.bacc as bacc
    import concourse.mybir as mybir
    from concourse.tile import TileContext
    from concourse.masks import make_identity

    fp16 = mybir.dt.float16
    fp32 = mybir.dt.float32

    nc = bacc.Bacc("TRN2", target_bir_lowering=False, debug=False)
    dense_t = nc.dram_tensor("dense", (bc, D), fp32, kind="ExternalInput")
    embs_t = nc.dram_tensor("embs", (bc, NUM_EMBS, D), fp32, kind="ExternalInput")
    out_t = nc.dram_tensor("out", (bc, NPAIR), fp32, kind="ExternalOutput")

    nblk = bc // BLK
    QBLK = 4             # blocks per quarter (pack-out granularity)
    nq = BLK // 4        # 32 4-sample groups per block
    ZT = 8               # groups per PSUM Z tile

    # Input load plan: small groups first (fast pipeline start).
    groups = []
    b = 0
    head = [1, 1, 2]
    while b < nblk:
        sz = min(head.pop(0) if head else 4, nblk - b)
        groups.append((b, sz))
        b += sz
    g_of = {}
    for gs, sz in groups:
        for i in range(sz):
            g_of[gs + i] = (gs, sz)

    # triu row offsets: off[m] = position of pair (m, m+1) in the packed row
    off = [0] * NF
    for m in range(1, NF):
        off[m] = off[m - 1] + (NF - 1 - (m - 1))

    with TileContext(nc) as tc:
        with (
            tc.tile_pool(name="consts", bufs=1) as consts,
            tc.tile_pool(name="xin", bufs=2) as xpool,
            tc.tile_pool(name="tt", bufs=4) as ttpool,
            tc.tile_pool(name="zs", bufs=2) as zspool,
            tc.tile_pool(name="tp", bufs=2, space="PSUM") as tppool,
            tc.tile_pool(name="zp", bufs=3, space="PSUM") as zppool,
        ):
            ident = consts.tile([128, 128], fp16)
            make_identity(nc, ident)

            dview = dense_t.ap()  # [bc, 128]
            eview = embs_t.ap().rearrange("b f d -> b (f d)")  # [bc, 3328]
            oview = out_t.ap()  # [bc, 351]

            X = None
            cp_i = 0  # round-robin counter for copy engines
            for qtr in range(nblk // QBLK):
                Zs = zspool.tile([128, QBLK * nq * NF], fp32, tag="Zs")
                Zsr = Zs.rearrange("p (t q n) -> p t q n", t=QBLK, q=nq)

                for blki in range(QBLK):
                    blk = qtr * QBLK + blki
                    gs, gsz = g_of[blk]
                    if blk == gs:
                        # SWDGE load casts fp32 -> fp16 at full rate
                        X = xpool.tile([BLK, gsz * NF * D], fp16, tag="X")
                        dsrc = dview[gs * BLK : (gs + gsz) * BLK].rearrange(
                            "(t b) d -> b t d", t=gsz
                        )  # [128, gsz, 128]
                        xd = X.rearrange("b (t c) -> b t c", t=gsz)
                        nc.gpsimd.dma_start(out=xd[:, :, 0:D], in_=dsrc)
                        esrc = eview[gs * BLK : (gs + gsz) * BLK].rearrange(
                            "(t b) c -> b t c", t=gsz
                        )  # [128, gsz, 3328]
                        nc.gpsimd.dma_start(out=xd[:, :, D:], in_=esrc)
                    xoff = (blk - gs) * NF * D

                    # ---- transpose 27 feature slabs into Tt [d, (b,f)] ----
                    # b-major with 32-feature pitch so a 4-sample weight
                    # group is 128 contiguous columns (walrus requires a
                    # single free dim on the stationary operand).
                    Tt = ttpool.tile([128, FP * D], fp16, tag="Tt")
                    Ttb = Tt.rearrange("d (b f) -> d b f", f=FP)  # [128,128,32]
                    nchunk = (NF + 7) // 8  # 8,8,8,3
                    for ci in range(nchunk):
                        c0 = ci * 8
                        cf = min(8, NF - c0)
                        tp = tppool.tile([128, 8 * BLK], fp16, tag="tp")
                        for j in range(cf):
                            f = c0 + j
                            nc.tensor.transpose(
                                tp[:, j * BLK : (j + 1) * BLK],
                                X[:, xoff + f * D : xoff + (f + 1) * D],
                                ident,
                            )
                        dst = Ttb[:, :, c0 : c0 + cf]
                        src = tp.rearrange("d (j b) -> d b j", b=BLK)[:, :, :cf]
                        if cp_i % 2 == 0:
                            nc.vector.tensor_copy(out=dst, in_=src)
                        else:
                            nc.scalar.copy(dst, src)
                        cp_i += 1

                    # ---- packed Gram matmuls: 4 samples per LDW+MM ----
                    for zt in range(nq // ZT):
                        zp = zppool.tile([128, ZT * 128], fp32, tag="zp")
                        zpc = zp.rearrange("p (q c) -> p q c", q=ZT)
                        for q in range(ZT):
                            qg = zt * ZT + q
                            wop = Tt[:, 4 * qg * FP : (4 * qg + 4) * FP]  # 128 cols
                            mop = Ttb[:, 4 * qg : 4 * qg + 4, :NF]        # 108 cols
                            nc.tensor.matmul(
                                zpc[:, q, : 4 * NF],
                                wop,
                                mop,
                                start=True,
                                stop=True,
                            )
                        # ---- diag 27x27 blocks -> Zs fp32 ----
                        for g in range(4):
                            src = zpc[32 * g : 32 * g + NF, :, NF * g : NF * (g + 1)]
                            dst = Zsr[
                                32 * g : 32 * g + NF, blki, zt * ZT : (zt + 1) * ZT, :
                            ]
                            if cp_i % 2 == 0:
                                nc.vector.tensor_copy(out=dst, in_=src)
                            else:
                                nc.scalar.copy(dst, src)
                            cp_i += 1

                # ---- triu pack straight to DRAM out (26 DMAs) ----
                Zs4 = Zs.rearrange(
                    "(gg m) (t q n) -> gg m t q n", gg=4, t=QBLK, q=nq
                )
                ovq = oview[qtr * QBLK * BLK : (qtr + 1) * QBLK * BLK].rearrange(
                    "(t q gg) p -> gg t q p", gg=4, q=nq
                )  # [4, 4, 32, 351]
                for m in range(NF - 1):
                    ln = NF - 1 - m
                    src = Zs4[:, m, :, :, m + 1 : NF]     # [4, 4, 32, ln]
                    dst = ovq[:, :, :, off[m] : off[m] + ln]
                    eng = nc.sync if m % 2 == 0 else nc.scalar
                    eng.dma_start(out=dst, in_=src)

    nc.compile()
    return nc


def _get(bc=BC):
    if bc not in _CACHE:
        _CACHE[bc] = build(bc)
    return _CACHE[bc]


def kernel(dense: np.ndarray, embs: np.ndarray) -> np.ndarray:
    from concourse import bass_utils

    dense = np.ascontiguousarray(np.asarray(dense, dtype=np.float32))
    embs = np.ascontiguousarray(np.asarray(embs, dtype=np.float32))
    assert dense.shape == (B, D) and embs.shape == (B, NUM_EMBS, D)

    nc = _get()
    dsh = dense.reshape(N_CORES, BC, D)
    esh = embs.reshape(N_CORES, BC, NUM_EMBS, D)
    in_maps = [{"dense": dsh[i], "embs": esh[i]} for i in range(N_CORES)]
    res = bass_utils.run_bass_kernel_spmd(nc, in_maps, core_ids=list(range(N_CORES)))
    return np.concatenate([r["out"] for r in res.results], axis=0)


# revision 5
# speedup vs baseline: 1.0086x; 1.0086x over previous
"""DLRM dot-interaction kernel for Trainium2 (8 NeuronCores, batch-sharded).

Per sample b: T = concat(dense[b], embs[b]) -> [27, 128]; Z = T @ T^T;
output = strict upper triangle of Z -> [351] fp32.

Per-core plan (2048 samples, 16 blocks of 128), v3:
  - SWDGE cast-DMA loads input blocks as [128 b, (f,d)] fp16.
  - PE transposes each [128 b, 128 d] feature slab into PSUM; copies land
    in b-major Tt [128 d, b*32+f] fp16 (32-feature pitch) so a 4-sample
    weight group is 128 contiguous columns.
  - Packed Gram matmuls: per 4-sample group one LDWEIGHTS (128 contiguous
    cols) + one 108-col matmul -> PSUM [128, 108-of-128]; diagonal 27x27
    blocks are the per-sample Grams.  Software-pipelined: transposes of
    block k interleave with Gram matmuls of block k-1 so the PE's HAM
    clock stays warm.
  - DVE/ACT/GPSIMD copy diag blocks to SBUF Zs [(g,m) part, (t,q,n)] fp32
    per half-core (8 blocks).
  - Triu pack straight to DRAM: per half 26 HWDGE DMAs (one per m, 2 free
    dims: [4 part, 256 rows, 26-m run]) scatter z[m, m+1:27] into
    out[s, off_m:]; no DRAM scratch bounce.
"""

import numpy as np

B, NUM_EMBS, D = 16384, 26, 128
N_CORES = 8
BC = B // N_CORES  # 2048 samples per core
BLK = 128          # samples per block
NF = NUM_EMBS + 1  # 27 features
FP = 32            # feature pitch in Tt (27 + 5 junk pad slots)
NPAIR = NF * (NF - 1) // 2  # 351

_CACHE = {}


def build(bc=BC):
    import concourse.bacc as bacc
    import concourse.mybir as mybir
    from concourse.tile import TileContext
    from concourse.masks import make_identity

    fp16 = mybir.dt.float16
    fp32 = mybir.dt.float32

    nc = bacc.Bacc("TRN2", target_bir_lowering=False, debug=False)
    dense_t = nc.dram_tensor("dense", (bc, D), fp32, kind="ExternalInput")
    embs_t = nc.dram_tensor("embs", (bc, NUM_EMBS, D), fp32, kind="ExternalInput")
    out_t = nc.dram_tensor("out", (bc, NPAIR), fp32, kind="ExternalOutput")

    nblk = bc // BLK
    HBLK = 8             # blocks per half (pack-out granularity)
    nq = BLK // 4        # 32 4-sample groups per block
    ZT = 8               # groups per PSUM Z tile

    groups = []
    b = 0
    head = [1, 1, 2]
    while b < nblk:
        sz = min(head.pop(0) if head else 4, nblk - b)
        groups.append((b, sz))
        b += sz
    g_of = {}
    for gs, sz in groups:
        for i in range(sz):
            g_of[gs + i] = (gs, sz)

    off = [0] * NF
    for m in range(1, NF):
        off[m] = off[m - 1] + (NF - m)

    chunks = [7, 7, 7, 6]  # feature chunks per transpose phase

    with TileContext(nc) as tc:
        with (
            tc.tile_pool(name="consts", bufs=1) as consts,
            tc.tile_pool(name="xin", bufs=2) as xpool,
            tc.tile_pool(name="tt", bufs=4) as ttpool,
            tc.tile_pool(name="zs", bufs=2) as zspool,
            tc.tile_pool(name="tp", bufs=2, space="PSUM") as tppool,
            tc.tile_pool(name="zp", bufs=3, space="PSUM") as zppool,
        ):
            ident = consts.tile([128, 128], fp16)
            make_identity(nc, ident)

            dview = dense_t.ap()  # [bc, 128]
            eview = embs_t.ap().rearrange("b f d -> b (f d)")  # [bc, 3328]
            oview = out_t.ap()  # [bc, 351]

            X = None
            Zs = None
            tts = {}   # live Tt tiles by block
            cp_i = 0   # round-robin copy-engine counter
            cp_engs = ("v", "s")  # gpsimd cannot access PSUM

            def do_copy(dst, src):
                nonlocal cp_i
                e = cp_engs[cp_i % len(cp_engs)]
                if e == "v":
                    nc.vector.tensor_copy(out=dst, in_=src)
                elif e == "s":
                    nc.scalar.copy(dst, src)
                else:
                    nc.gpsimd.tensor_copy(out=dst, in_=src)
                cp_i += 1

            def pack_out(h, Zs_h):
                # 26 HWDGE DMAs: triu-pack straight to DRAM out rows
                Zr = Zs_h.rearrange("(gg m) (tq n) -> gg m tq n", gg=4, n=NF)
                ovh = oview[h * HBLK * BLK : (h + 1) * HBLK * BLK].rearrange(
                    "(tq gg) p -> gg tq p", gg=4
                )  # [4, 256, 351]
                for m in range(NF - 1):
                    ln = NF - 1 - m
                    src = Zr[:, m, :, m + 1 : NF]          # [4, 256, ln]
                    dst = ovh[:, :, off[m] : off[m] + ln]  # [4, 256, ln]
                    eng = nc.sync if m % 2 == 0 else nc.scalar
                    eng.dma_start(out=dst, in_=src)

            for blk in range(nblk + 1):
                if blk < nblk and blk % HBLK == 0:
                    Zs_new = zspool.tile([128, HBLK * nq * NF], fp32, tag="Zs")
                if blk < nblk:
                    gs, gsz = g_of[blk]
                    if blk == gs:
                        X = xpool.tile([BLK, gsz * NF * D], fp16, tag="X")
                        dsrc = dview[gs * BLK : (gs + gsz) * BLK].rearrange(
                            "(t b) d -> b t d", t=gsz
                        )
                        xd = X.rearrange("b (t c) -> b t c", t=gsz)
                        nc.gpsimd.dma_start(out=xd[:, :, 0:D], in_=dsrc)
                        esrc = eview[gs * BLK : (gs + gsz) * BLK].rearrange(
                            "(t b) c -> b t c", t=gsz
                        )
                        nc.gpsimd.dma_start(out=xd[:, :, D:], in_=esrc)
                    xoff = (blk - gs) * NF * D
                    Tt = ttpool.tile([128, BLK * FP], fp16, tag="Tt")
                    Ttb = Tt.rearrange("d (b f) -> d b f", f=FP)
                    tts[blk] = (Tt, Ttb)

                # interleave: transpose chunk p of blk with Gram zt=p of blk-1
                c0 = 0
                for phase in range(4):
                    cf = chunks[phase]
                    if blk < nblk:
                        tp = tppool.tile([128, 8 * BLK], fp16, tag="tp")
                        for j in range(cf):
                            f = c0 + j
                            nc.tensor.transpose(
                                tp[:, j * BLK : (j + 1) * BLK],
                                X[:, xoff + f * D : xoff + (f + 1) * D],
                                ident,
                            )
                        dst = Ttb[:, :, c0 : c0 + cf]
                        src = tp.rearrange("d (j b) -> d b j", b=BLK)[:, :, :cf]
                        do_copy(dst, src)
                    if blk > 0:
                        kk = blk - 1
                        TtK, TtbK = tts[kk]
                        zp = zppool.tile([128, ZT * 128], fp32, tag="zp")
                        zpc = zp.rearrange("p (q c) -> p q c", q=ZT)
                        for q in range(ZT):
                            qg = phase * ZT + q
                            wop = TtK[:, 4 * qg * FP : (4 * qg + 4) * FP]
                            mop = TtbK[:, 4 * qg : 4 * qg + 4, :NF]
                            nc.tensor.matmul(
                                zpc[:, q, : 4 * NF], wop, mop,
                                start=True, stop=True,
                            )
                        Zsr = Zs.rearrange(
                            "p (t q n) -> p t q n", t=HBLK, q=nq
                        )
                        for g in range(4):
                            srcz = zpc[
                                32 * g : 32 * g + NF, :, NF * g : NF * (g + 1)
                            ]
                            dstz = Zsr[
                                32 * g : 32 * g + NF,
                                kk % HBLK,
                                phase * ZT : (phase + 1) * ZT,
                                :,
                            ]
                            do_copy(dstz, srcz)
                    c0 += cf

                if blk > 0 and (blk - 1) % HBLK == HBLK - 1:
                    pack_out((blk - 1) // HBLK, Zs)
                    tts.pop(blk - 2, None)
                if blk < nblk and blk % HBLK == 0:
                    Zs = Zs_new
                # free old Tt refs (pool rotation handles actual reuse)
                tts.pop(blk - 2, None)

    nc.compile()
    return nc


def _get(bc=BC):
    if bc not in _CACHE:
        _CACHE[bc] = build(bc)
    return _CACHE[bc]


def kernel(dense: np.ndarray, embs: np.ndarray) -> np.ndarray:
    from concourse import bass_utils

    dense = np.ascontiguousarray(np.asarray(dense, dtype=np.float32))
    embs = np.ascontiguousarray(np.asarray(embs, dtype=np.float32))
    assert dense.shape == (B, D) and embs.shape == (B, NUM_EMBS, D)

    nc = _get()
    dsh = dense.reshape(N_CORES, BC, D)
    esh = embs.reshape(N_CORES, BC, NUM_EMBS, D)
    in_maps = [{"dense": dsh[i], "embs": esh[i]} for i in range(N_CORES)]
    res = bass_utils.run_bass_kernel_spmd(nc, in_maps, core_ids=list(range(N_CORES)))
    return np.concatenate([r["out"] for r in res.results], axis=0)
